# revision 4
# baseline (speedup 1.0000x reference)
"""Trainium2 Bass kernel for nn_DepthwiseMultiKernelAttention.

out = softmax_rows(G) @ P @ softmax_rows(A) @ depthwise3x3(x, K) folds to
out = sum_s M9[s] @ shift_s(x) with M9[s] = M*diag(K[:,s]), M = Gsm@P@Asm
(all host-folded). 8 cores: core i = (sample pair i//2, row half i%2);
2 samples x 64 ch packed in the 128 SBUF partitions, host pre-pads halos.

f16 end-to-end (DMA ~52us on the shared device; DVE 2x/4x perf modes;
rel err ~7e-4). Per 16-row chunk, per 8-row group:
  PE:   taps 0,1,2,8 as blockdiag matmuls per 2-row psum tile + 2 mix folds
  DVE:  taps 3,4,5 as ts,ts,tt,ts,tt chain (ts@4x, tt@2x) + ts tap 7
  Act:  tap 6 via activation(scale) + psum->sbuf f32->f16 drains (2-row)
  DMA:  SWDGE accumulate (t6 += t7) merges chain2 on the DMA engines --
        Pool pays only the ~1us descriptor issue, transfers pipeline on
        the half-idle DMA device (Pool tensor_add was the serial spine)
Vector chains run 2 chunks ahead of PE; folds deferred 4 tiles; drains
attach to their tile's fold (emission order = tile-framework dependency
order: a drain emitted before its fold reads pre-fold psum) and release
1-bank psum slots. Stores ride SP HWDGE per chunk; tail chunk drains
and stores at 2-row grain alternating Act/DVE.
TimelineSim (the graded metric): ~92.7us vs 103.5us baseline.
"""

import numpy as np

B, C, H, W = 8, 64, 256, 256
N_CORES = 8
HH = H // 2
PR, PC = HH + 2, W + 2
RPC = 16           # rows per chunk
N_CHUNKS = HH // RPC
GR = 8             # rows per vector group / psum tile
RPP = 2            # rows per matmul (one psum bank)
SHIFTS = [(dy, dx) for dy in range(3) for dx in range(3)]

PE_TAPS = (0, 1, 2)      # always-PE taps
DVE_TAPS = (3, 4, 5)     # DVE ts/tt chain
ACT_TAP = 6              # Act activation tap (chain2 base)
POOL_TAP = 7             # Pool stt tap (merges Act product)
TAP8 = 8

# per-group assignment of tap 8: 'pe' | 'pool' | 'dve'
N_GROUPS = N_CHUNKS * (RPC // GR)  # 16
DEFAULT_TAP8 = tuple(('pe', 'pool')[g % 2] for g in range(N_GROUPS))


def tap8_mix(pat):
    """Cyclic tap8 schedule from a pattern string like 'pe,pool,dve,pool'."""
    names = pat.split(',')
    return tuple(names[g % len(names)] for g in range(N_GROUPS))

WSLOT = {0: 0, 1: 1, 2: 2, TAP8: 3, ACT_TAP: 5, POOL_TAP: 6}
MIX_SLOT = 4
NW = 7

LAST_EXEC_NS = None
_PROGRAM = None


BEST_TAP8 = tap8_mix('pe')
DR_PAIRS = ((0, 1),)


def _build_program(tap8=BEST_TAP8, defer=4, x_bufs=4, o_bufs=2,
                   acc_bufs=6, tmp_bufs=2, t6_bufs=6, warmup_mm=4,
                   split_first=True, store_eng='sync', drain_defer=2,
                   ahead=2, prefetch_at='top', chain2_pe=(0,),
                   tail_dve_drains=True, chains16=False, ps_rows=2,
                   halo_dma=False, warmup_memset=True, first_piece=6,
                   chain2_dve=(), merge_dma=(), merge67='dma',
                   chain2_16=False, fold_merge=None, split_wt=False, act16=False,
                   split_chunks=(1,), dr_pairs=(), tap7_pe=False,
                   store_halves=False):
    import concourse.mybir as mybir
    import concourse.tile as tile
    from concourse import bacc

    f32 = mybir.dt.float32
    f16 = mybir.dt.float16
    mult = mybir.AluOpType.mult
    add = mybir.AluOpType.add
    Copy = mybir.ActivationFunctionType.Copy

    nc = bacc.Bacc("TRN2", target_bir_lowering=False, debug=False,
                   num_devices=N_CORES)
    x_d = nc.dram_tensor("xs", [128, PR, PC], f16, kind="ExternalInput").ap()
    w_d = nc.dram_tensor("wt", [128, NW, 128], f16, kind="ExternalInput").ap()
    k_d = nc.dram_tensor("kv", [128, 9], f32, kind="ExternalInput").ap()
    f8 = mybir.dt.float8e4
    x8_d = nc.dram_tensor("x8", [128, PR, PC], f8, kind="ExternalInput").ap()
    w8_d = nc.dram_tensor("w8", [128, 2, 2, 128], f8,
                          kind="ExternalInput").ap()
    o_d = nc.dram_tensor("out", [128, HH, W], f16, kind="ExternalOutput").ap()

    with tile.TileContext(nc) as tc:
        with (
            tc.tile_pool(name="wpool", bufs=1) as wpool,
            tc.tile_pool(name="xpool", bufs=x_bufs) as xpool,
            tc.tile_pool(name="ppool", bufs=16 // ps_rows,
                         space="PSUM") as ppool,
            tc.tile_pool(name="opool", bufs=o_bufs) as opool,
            tc.tile_pool(name="vpool", bufs=2) as vpool,
        ):
            # PE warmup: fills initial DMA wait so p-state is ramped.
            # warmup matmuls read uninitialized SBUF: values never escape
            # (every real accumulation group opens with start=True)
            scratch = wpool.tile([128, 512], f32, tag="scratch")
            if warmup_memset == 'dve':
                nc.vector.memset(scratch, 0.0)
            elif warmup_memset:
                nc.gpsimd.memset(scratch, 0.0)
            sc16 = scratch.bitcast(mybir.dt.bfloat16)
            wps = ppool.tile([128, ps_rows, W], f32, name="ps", tag="ps")
            for _ in range(warmup_mm):
                nc.tensor.matmul(wps[:, :2, :],
                                 lhsT=sc16[:, :128], rhs=sc16[:, :512],
                                 start=True, stop=True)

            wt = wpool.tile([128, NW, 128], f16)
            if split_wt:
                # early slots first so the first tap matmuls aren't gated on
                # the full weight block
                nc.scalar.dma_start(out=wt[:, :3, :], in_=w_d[:, :3, :])
                nc.scalar.dma_start(out=wt[:, 3:, :], in_=w_d[:, 3:, :])
            else:
                nc.scalar.dma_start(out=wt, in_=w_d)
            kv = wpool.tile([128, 9], f32)
            nc.scalar.dma_start(out=kv, in_=k_d)
            w8t = None
            if dr_pairs:
                w8t = wpool.tile([128, 2, 2, 128], f8)
                nc.scalar.dma_start(out=w8t, in_=w8_d)

            first = xpool.tile([128, RPC + 2, PC], f16, name="xt", tag="xt")
            x8ts = {}
            if dr_pairs:
                fx8 = xpool.tile([128, RPC + 2, PC], f8, name="x8t", tag="x8t")
                nc.scalar.dma_start(out=fx8[:, :10, :],
                                     in_=x8_d[:, :10, :])
                nc.scalar.dma_start(out=fx8[:, 10:, :],
                                    in_=x8_d[:, 10:RPC + 2, :])
                x8ts[0] = fx8
            first_tail_dma = None
            if split_first:
                r = 0
                while r < RPC + 2:
                    e = min(r + first_piece, RPC + 2)
                    if RPC + 2 - e < 4:
                        e = RPC + 2
                    nc.sync.dma_start(out=first[:, r:e, :],
                                      in_=x_d[:, r:e, :])
                    r = e
            else:
                nc.sync.dma_start(out=first, in_=x_d[:, :RPC + 2, :])

            # deferred PE folds: (ps_slice, acc1_sl, acc2_sl)
            pending = []
            # deferred Act drains: (chunk, ot_slice, ps8)
            drains = []
            drained = {}   # chunk -> number of drains emitted
            store_info = {}  # chunk -> (dram_slice, ot)
            store_eng_obj = None  # set below

            def flush_folds(limit=0):
                # a tile's drain is only queued once its folds are emitted —
                # otherwise the tile framework orders the drain BEFORE the
                # fold (it reads the psum's pre-fold state): wrong results
                while len(pending) > limit:
                    ps2, a1, a2, drain_info = pending.pop(0)
                    nc.tensor.matmul(ps2, lhsT=wt[:, MIX_SLOT, :], rhs=a1,
                                     start=False, stop=a2 is None,
                                     skip_group_check=bool(dr_pairs))
                    if a2 is not None:
                        nc.tensor.matmul(ps2, lhsT=wt[:, MIX_SLOT, :], rhs=a2,
                                         start=False, stop=True,
                                         skip_group_check=bool(dr_pairs))
                    if drain_info is not None:
                        drains.append(drain_info)

            def flush_drains(limit=0):
                while len(drains) > limit:
                    ck, dst, ps8 = drains.pop(0)
                    nc.scalar.copy(out=dst, in_=ps8)
                    drained[ck] = drained.get(ck, 0) + 1
                    half = RPC // ps_rows // 2
                    if store_halves and drained[ck] == half:
                        dsl, ot_t = store_info[ck]
                        store_eng_obj.dma_start(
                            out=dsl[:, :RPC // 2, :],
                            in_=ot_t[:, :RPC // 2, :])
                    elif drained[ck] == RPC // ps_rows:
                        dsl, ot_t = store_info.pop(ck)
                        if store_halves:
                            store_eng_obj.dma_start(
                                out=dsl[:, RPC // 2:, :],
                                in_=ot_t[:, RPC // 2:, :])
                        else:
                            store_eng_obj.dma_start(out=dsl, in_=ot_t)

            def drain_fine(ck, ot_t, ps8, rg, done_tiles):
                """2-row drains + store slices for the tail (latency);
                alternate Act/DVE so the tail is not Act-serial."""
                for t in done_tiles:
                    sl = slice(rg + RPP * t, rg + RPP * (t + 1))
                    use_dve = (tail_dve_drains is True and
                               (rg // RPP + t) % 2 == 1)
                    if tail_dve_drains == 'last' and rg + RPP * t >= RPC - 6:
                        use_dve = (rg // RPP + t) % 2 == 1
                    if use_dve:
                        nc.vector.tensor_copy(ot_t[:, sl, :],
                                              ps8[:, RPP * t:RPP * (t + 1), :])
                    else:
                        nc.scalar.copy(out=ot_t[:, sl, :],
                                       in_=ps8[:, RPP * t:RPP * (t + 1), :])
                    r0 = ck * RPC
                    store_eng_obj.dma_start(
                        out=o_d[:, r0 + sl.start:r0 + sl.stop, :],
                        in_=ot_t[:, sl, :])

            def emit_chains16(chunk, xt_t):
                """Whole-chunk (16-row) vector chains: fewer, bigger ops."""
                g = chunk * (RPC // GR)  # use first group's tap8 assignment

                def xs16(si):
                    dy, dx = SHIFTS[si]
                    return xt_t[:, dy:dy + RPC, dx:dx + W]

                t3 = vpool.tile([128, RPC, W], f16, name="t3", tag="t3",
                                bufs=tmp_bufs)
                nc.vector.tensor_scalar(t3, xs16(DVE_TAPS[0]),
                                        kv[:, DVE_TAPS[0]:DVE_TAPS[0] + 1],
                                        None, mult)
                t4 = vpool.tile([128, RPC, W], f16, name="t4", tag="t4",
                                bufs=tmp_bufs)
                nc.vector.tensor_scalar(t4, xs16(DVE_TAPS[1]),
                                        kv[:, DVE_TAPS[1]:DVE_TAPS[1] + 1],
                                        None, mult)
                acc1 = vpool.tile([128, RPC, W], f16, name="acc1", tag="acc1",
                                  bufs=acc_bufs)
                nc.vector.tensor_tensor(acc1, t3, t4, add)
                t5 = vpool.tile([128, RPC, W], f16, name="t5", tag="t5",
                                bufs=tmp_bufs)
                nc.vector.tensor_scalar(t5, xs16(DVE_TAPS[2]),
                                        kv[:, DVE_TAPS[2]:DVE_TAPS[2] + 1],
                                        None, mult)
                nc.vector.tensor_tensor(acc1, acc1, t5, add)
                if tap8[g] == 'dve' and g not in chain2_pe:
                    t8 = vpool.tile([128, RPC, W], f16, name="t8", tag="t8",
                                    bufs=tmp_bufs)
                    nc.vector.tensor_scalar(t8, xs16(TAP8),
                                            kv[:, TAP8:TAP8 + 1], None, mult)
                    nc.vector.tensor_tensor(acc1, acc1, t8, add)
                if g in chain2_pe:
                    return {s: (acc1[:, GR * s:GR * (s + 1), :], None, True)
                            for s in range(RPC // GR)}
                t6 = vpool.tile([128, RPC, W], f16, name="t6", tag="t6",
                                bufs=t6_bufs)
                nc.scalar.activation(out=t6, in_=xs16(ACT_TAP), func=Copy,
                                     scale=kv[:, ACT_TAP:ACT_TAP + 1])
                acc2 = vpool.tile([128, RPC, W], f16, name="acc2", tag="acc2",
                                  bufs=acc_bufs)
                nc.gpsimd.scalar_tensor_tensor(
                    out=acc2, in0=xs16(POOL_TAP),
                    scalar=kv[:, POOL_TAP:POOL_TAP + 1], in1=t6,
                    op0=mult, op1=add)
                if tap8[g] == 'pool':
                    nc.gpsimd.scalar_tensor_tensor(
                        out=acc2, in0=xs16(TAP8), scalar=kv[:, TAP8:TAP8 + 1],
                        in1=acc2, op0=mult, op1=add)
                pe8 = tap8[g] == 'pe'
                return {s: (acc1[:, GR * s:GR * (s + 1), :],
                            acc2[:, GR * s:GR * (s + 1), :], pe8)
                        for s in range(RPC // GR)}

            def emit_chain2_16(chunk, xt_t):
                """Chunk-granularity chain2: act6 + ts7 + Pool tt merge over
                16 rows (amortizes per-op init overheads)."""
                def xs16(si):
                    dy, dx = SHIFTS[si]
                    return xt_t[:, dy:dy + RPC, dx:dx + W]

                t6 = vpool.tile([128, RPC, W], f16, name="t6", tag="t6",
                                bufs=3)
                nc.scalar.activation(out=t6, in_=xs16(ACT_TAP), func=Copy,
                                     scale=kv[:, ACT_TAP:ACT_TAP + 1])
                t7 = vpool.tile([128, RPC, W], f16, name="t7", tag="t7",
                                bufs=3)
                nc.vector.tensor_scalar(t7, xs16(POOL_TAP),
                                        kv[:, POOL_TAP:POOL_TAP + 1],
                                        None, mult)
                acc2 = vpool.tile([128, RPC, W], f16, name="acc2",
                                  tag="acc2", bufs=3)
                if merge67 == 'pool':
                    nc.gpsimd.tensor_add(acc2, t6, t7)
                else:
                    nc.vector.tensor_add(acc2, t6, t7)
                return acc2

            def emit_chains(chunk, xt_t):
                """Vector-engine tap chains for both groups of a chunk."""
                if chains16:
                    return emit_chains16(chunk, xt_t)
                acc2_16 = None
                t6_16 = None
                if chain2_16:
                    g0 = chunk * (RPC // GR)
                    if (g0 not in chain2_pe and g0 not in chain2_dve):
                        acc2_16 = emit_chain2_16(chunk, xt_t)
                elif act16:
                    g0 = chunk * (RPC // GR)
                    if (g0 not in chain2_pe and g0 not in chain2_dve
                            and (g0 + 1) not in chain2_pe):
                        # 16-row act6 (amortized init); per-group merges
                        dy, dx = SHIFTS[ACT_TAP]
                        t6_16 = vpool.tile([128, RPC, W], f16, name="t616",
                                           tag="t616", bufs=3)
                        nc.scalar.activation(
                            out=t6_16, in_=xt_t[:, dy:dy + RPC, dx:dx + W],
                            func=Copy, scale=kv[:, ACT_TAP:ACT_TAP + 1])
                out = {}
                for sub in range(RPC // GR):
                    g = chunk * (RPC // GR) + sub
                    rg = GR * sub

                    def xs8(si):
                        dy, dx = SHIFTS[si]
                        return xt_t[:, rg + dy:rg + dy + GR, dx:dx + W]

                    # chain1 on DVE: taps 3,4,5 (+ tap8 if assigned)
                    t3 = vpool.tile([128, GR, W], f16, name="t3", tag="t3",
                                    bufs=tmp_bufs)
                    nc.vector.tensor_scalar(t3, xs8(DVE_TAPS[0]),
                                            kv[:, DVE_TAPS[0]:DVE_TAPS[0] + 1],
                                            None, mult)
                    t4 = vpool.tile([128, GR, W], f16, name="t4", tag="t4",
                                    bufs=tmp_bufs)
                    nc.vector.tensor_scalar(t4, xs8(DVE_TAPS[1]),
                                            kv[:, DVE_TAPS[1]:DVE_TAPS[1] + 1],
                                            None, mult)
                    a34 = vpool.tile([128, GR, W], f16, name="a34", tag="a34",
                                     bufs=tmp_bufs)
                    nc.vector.tensor_tensor(a34, t3, t4, add)
                    t5 = vpool.tile([128, GR, W], f16, name="t5", tag="t5",
                                    bufs=tmp_bufs)
                    nc.vector.tensor_scalar(t5, xs8(DVE_TAPS[2]),
                                            kv[:, DVE_TAPS[2]:DVE_TAPS[2] + 1],
                                            None, mult)
                    if (tap8[g] == 'dve' and g not in chain2_pe
                            and g not in chain2_dve):
                        a345 = vpool.tile([128, GR, W], f16, name="a345",
                                          tag="a345", bufs=tmp_bufs)
                        nc.vector.tensor_tensor(a345, a34, t5, add)
                        t8 = vpool.tile([128, GR, W], f16, name="t8", tag="t8",
                                        bufs=tmp_bufs)
                        nc.vector.tensor_scalar(t8, xs8(TAP8),
                                                kv[:, TAP8:TAP8 + 1],
                                                None, mult)
                        acc1 = vpool.tile([128, GR, W], f16, name="acc1",
                                          tag="acc1", bufs=acc_bufs)
                        nc.vector.tensor_tensor(acc1, a345, t8, add)
                    else:
                        acc1 = vpool.tile([128, GR, W], f16, name="acc1",
                                          tag="acc1", bufs=acc_bufs)
                        nc.vector.tensor_tensor(acc1, a34, t5, add)

                    if g in chain2_pe:
                        # startup/tail groups: chain2 taps 6,7 ride PE (no
                        # Act/Pool latency in the critical path); tap8 per
                        # its own assignment ('dve' folds into chain1)
                        out[sub] = (acc1, None, tap8[g] != 'dve')
                        continue
                    if g in chain2_dve:
                        # chain2 folded into the DVE chain (ts+tt per tap)
                        for si in (ACT_TAP, POOL_TAP, TAP8):
                            tx = vpool.tile([128, GR, W], f16, name="tx",
                                            tag="tx", bufs=tmp_bufs)
                            nc.vector.tensor_scalar(tx, xs8(si),
                                                    kv[:, si:si + 1],
                                                    None, mult)
                            na = vpool.tile([128, GR, W], f16, name="acc1",
                                            tag="acc1", bufs=acc_bufs)
                            nc.vector.tensor_tensor(na, acc1, tx, add)
                            acc1 = na
                        out[sub] = (acc1, None, False)
                        continue
                    if acc2_16 is not None:
                        out[sub] = (acc1,
                                    acc2_16[:, rg:rg + GR, :],
                                    tap8[g] != 'dve')
                        continue
                    # chain2: Act tap6 + DVE ts tap7, merged by a Pool
                    # tensor_tensor add (Pool can't run TensorScalarPtr)
                    if t6_16 is not None:
                        t6 = t6_16[:, rg:rg + GR, :]
                    else:
                        t6 = vpool.tile([128, GR, W], f16, name="t6",
                                        tag="t6", bufs=t6_bufs)
                        nc.scalar.activation(out=t6, in_=xs8(ACT_TAP),
                                             func=Copy,
                                             scale=kv[:, ACT_TAP:ACT_TAP + 1])
                    t7 = vpool.tile([128, GR, W], f16, name="t7", tag="t7",
                                    bufs=t6_bufs)
                    nc.vector.tensor_scalar(t7, xs8(POOL_TAP),
                                            kv[:, POOL_TAP:POOL_TAP + 1],
                                            None, mult)
                    m67 = merge67
                    if merge67 == 'alt':
                        m67 = 'pool' if g % 2 == 0 else 'dve'
                    elif merge67 == 'alt4':
                        m67 = 'dve' if g % 4 == 3 else 'pool'
                    if m67 == 'dma':
                        # SWDGE accumulate: t6 += t7 on the DMA engines;
                        # Pool only pays the ~1us descriptor-gen issue
                        nc.gpsimd.dma_start(out=t6, in_=t7,
                                            accum_op=add)
                        acc2 = t6
                    elif True:
                        acc2 = vpool.tile([128, GR, W], f16, name="acc2",
                                          tag="acc2", bufs=acc_bufs)
                    if m67 == 'dma':
                        pass
                    elif m67 == 'pool':
                        nc.gpsimd.tensor_add(acc2, t6, t7)
                    else:
                        nc.vector.tensor_add(acc2, t6, t7)
                    if g in merge_dma:
                        # fold acc2 into acc1 on the DMA engines (CCE
                        # accumulate) — saves one PE mix-matmul per tile
                        nc.gpsimd.dma_start(out=acc1, in_=acc2,
                                            accum_op=add)
                        out[sub] = (acc1, None, tap8[g] != 'dve')
                    elif fold_merge and fold_merge[g]:
                        # merge acc1+acc2 on a vector engine: one PE fold
                        # per tile instead of two
                        accm = vpool.tile([128, GR, W], f16, name="accm",
                                          tag="accm", bufs=acc_bufs)
                        if fold_merge[g] == 'pool':
                            nc.gpsimd.tensor_add(accm, acc1, acc2)
                        else:
                            nc.vector.tensor_add(accm, acc1, acc2)
                        out[sub] = (accm, None, tap8[g] != 'dve')
                    else:
                        out[sub] = (acc1, acc2, tap8[g] != 'dve')
                return out

            store_eng_obj = {'sync': nc.sync, 'gpsimd': nc.gpsimd,
                             'scalar': nc.scalar}[store_eng]
            xts = {0: first}
            accs = {0: emit_chains(0, first)}

            def prefetch(c):
                if c >= N_CHUNKS or c in xts:
                    return
                r0 = c * RPC
                nxt = xpool.tile([128, RPC + 2, PC], f16, name="xt",
                                 tag="xt")
                if halo_dma:
                    # halo rows re-read from HBM (DMA has slack; frees DVE
                    # and decouples this tile from the previous one)
                    nc.sync.dma_start(out=nxt,
                                      in_=x_d[:, r0:r0 + RPC + 2, :])
                else:
                    # halo rows from previous chunk's tile (DVE 4x copy)
                    nc.vector.tensor_copy(nxt[:, 0:2, :],
                                          xts[c - 1][:, RPC:RPC + 2, :])
                    if c in split_chunks:
                        # two pieces: the chunk's chain2 (rows 2..12) can
                        # start ~1.5us before the full chunk lands
                        nc.sync.dma_start(out=nxt[:, 2:12, :],
                                          in_=x_d[:, r0 + 2:r0 + 12, :])
                        nc.sync.dma_start(out=nxt[:, 12:, :],
                                          in_=x_d[:, r0 + 12:r0 + RPC + 2, :])
                    else:
                        nc.sync.dma_start(out=nxt[:, 2:, :],
                                          in_=x_d[:, r0 + 2:r0 + RPC + 2, :])
                if dr_pairs:
                    nx8 = xpool.tile([128, RPC + 2, PC], f8, name="x8t",
                                     tag="x8t")
                    nc.scalar.dma_start(out=nx8,
                                         in_=x8_d[:, r0:r0 + RPC + 2, :])
                    x8ts[c] = nx8
                xts[c] = nxt
                accs[c] = emit_chains(c, nxt)

            for c in range(1, ahead):
                prefetch(c)
                if c == 1 and first_tail_dma is not None:
                    first_tail_dma()
                    first_tail_dma = None
            if first_tail_dma is not None:
                first_tail_dma()
                first_tail_dma = None
            for chunk in range(N_CHUNKS):
                xt = xts[chunk]
                if prefetch_at == 'top':
                    prefetch(chunk + ahead)

                ot = opool.tile([128, RPC, W], f16)
                last = chunk == N_CHUNKS - 1
                if last:
                    flush_drains(0)
                else:
                    store_info[chunk] = (
                        o_d[:, chunk * RPC:chunk * RPC + RPC, :], ot)
                # PE loop over 2-row tiles, psum tiles of ps_rows rows
                for sub in range(RPC // GR):
                    rg = GR * sub
                    acc1, acc2, pe_tap8 = accs[chunk][sub]
                    for pt in range(GR // ps_rows):
                        rp = rg + ps_rows * pt
                        psT = ppool.tile([128, ps_rows, W], f32, name="ps",
                                         tag="ps")
                        for t in range(ps_rows // RPP):
                            rr = rp + RPP * t

                            def xs2(si):
                                dy, dx = SHIFTS[si]
                                return xt[:, rr + dy:rr + dy + RPP,
                                          dx:dx + W]

                            ps2 = psT[:, RPP * t:RPP * (t + 1), :]
                            pe_sis = list(PE_TAPS)
                            if acc2 is None:
                                pe_sis += [ACT_TAP, POOL_TAP]
                            if pe_tap8:
                                pe_sis.append(TAP8)
                            dr_cov = set()
                            for (sa, sb) in dr_pairs:
                                dr_cov |= {sa, sb}
                            pe_sis = [s_ for s_ in pe_sis
                                      if s_ not in dr_cov]
                            # fp8 DoubleRow: one half-cost matmul per psum
                            # row computes BOTH taps of a pair (the pair dim
                            # is a stride-delta view into the fp8 x copy)
                            x8t = x8ts.get(chunk)
                            for i, si in enumerate(pe_sis):
                                nc.tensor.matmul(ps2,
                                                 lhsT=wt[:, WSLOT[si], :],
                                                 rhs=xs2(si),
                                                 start=(i == 0),
                                                 stop=False,
                                                 skip_group_check=bool(
                                                     dr_pairs))
                            for p, (sa, sb) in enumerate(dr_pairs):
                                dya, dxa = SHIFTS[sa]
                                dyb, dxb = SHIFTS[sb]
                                delta = (dyb - dya) * PC + (dxb - dxa)
                                for r_ in range(RPP):
                                    base = x8t[:, rr + dya + r_,
                                               dxa:dxa + W]
                                    rhs8 = type(base)(
                                        tensor=base.tensor,
                                        offset=base.offset,
                                        ap=[base.ap[0], [delta, 2], [1, W]])
                                    nc.tensor.matmul(
                                        psT[:, RPP * t + r_:
                                            RPP * t + r_ + 1, :],
                                        lhsT=w8t[:, p, :, :], rhs=rhs8,
                                        start=False, stop=False,
                                        perf_mode=(
                                            mybir.MatmulPerfMode.DoubleRow),
                                        skip_group_check=True)
                            sl = slice(rr - rg, rr - rg + RPP)
                            a2sl = None if acc2 is None else acc2[:, sl, :]
                            if last:
                                flush_folds(limit=0)
                                nc.tensor.matmul(ps2, lhsT=wt[:, MIX_SLOT, :],
                                                 rhs=acc1[:, sl, :],
                                                 start=False,
                                                 stop=a2sl is None,
                                                 skip_group_check=bool(
                                                     dr_pairs))
                                if a2sl is not None:
                                    nc.tensor.matmul(
                                        ps2, lhsT=wt[:, MIX_SLOT, :],
                                        rhs=a2sl, start=False, stop=True,
                                        skip_group_check=bool(dr_pairs))
                                drain_fine(chunk, ot, psT, rp, [t])
                            else:
                                flush_folds(limit=defer)
                                dinfo = None
                                if t == ps_rows // RPP - 1:
                                    dinfo = (chunk,
                                             ot[:, rp:rp + ps_rows, :], psT)
                                pending.append(
                                    (ps2, acc1[:, sl, :], a2sl, dinfo))
                        if not last:
                            flush_drains(limit=drain_defer)
                if prefetch_at == 'bottom':
                    prefetch(chunk + ahead)
            flush_folds(0)
            flush_drains(0)
    nc.compile()
    return nc


def _get_program():
    global _PROGRAM
    if _PROGRAM is None:
        _PROGRAM = _build_program()
    return _PROGRAM


def _softmax_rows(a):
    a = a.astype(np.float64)
    a = np.exp(a - a.max(axis=1, keepdims=True))
    return a / a.sum(axis=1, keepdims=True)


def _make_weights(depthwise_weights, pointwise_weights, attention_weights,
                  global_attention_weight):
    A = _softmax_rows(np.asarray(attention_weights))
    G = _softmax_rows(np.asarray(global_attention_weight))
    P = np.asarray(pointwise_weights)[:, :, 0, 0].astype(np.float64)
    M = G @ P @ A
    Kdw = np.asarray(depthwise_weights)[:, 0].astype(np.float64)  # (64,3,3)
    wt = np.zeros((128, NW, 128), np.float16)
    for si in list(WSLOT):
        dy, dx = SHIFTS[si]
        blk = (M.T * Kdw[:, dy, dx][:, None]).astype(np.float16)
        wt[:C, WSLOT[si], :C] = blk
        wt[C:, WSLOT[si], C:] = blk
    mixT = M.T.astype(np.float16)
    wt[:C, MIX_SLOT, :C] = mixT
    wt[C:, MIX_SLOT, C:] = mixT
    kva = np.empty((128, 9), np.float32)
    for si, (dy, dx) in enumerate(SHIFTS):
        kva[:C, si] = Kdw[:, dy, dx]
        kva[C:, si] = Kdw[:, dy, dx]
    # fp8 DoubleRow pair weights
    import concourse.mybir as mybir
    f8np = mybir.dt.np(mybir.dt.float8e4)
    w8 = np.zeros((128, 2, 2, 128), f8np)
    for p, (sa, sb) in enumerate(DR_PAIRS):
        for i, si in enumerate((sa, sb)):
            dy, dx = SHIFTS[si]
            blk = (M.T * Kdw[:, dy, dx][:, None]).astype(f8np)
            w8[:C, p, i, :C] = blk
            w8[C:, p, i, C:] = blk
    return wt, kva, w8


def _make_shards(x):
    x = np.asarray(x, dtype=np.float32)
    shards = []
    for i in range(N_CORES):
        p, h = divmod(i, 2)
        buf = np.zeros((2, C, PR, PC), np.float16)
        r0 = HH * h - 1
        r1 = HH * h + HH + 1
        sr0, sr1 = max(r0, 0), min(r1, H)
        buf[:, :, sr0 - r0:sr1 - r0, 1:1 + W] = (
            x[2 * p:2 * p + 2, :, sr0:sr1, :].astype(np.float16))
        shards.append(buf.reshape(128, PR, PC))
    return shards


def _make_shards8(shards):
    import concourse.mybir as mybir
    f8np = mybir.dt.np(mybir.dt.float8e4)
    return [s.astype(f8np) for s in shards]


def kernel(x, depthwise_weights, pointwise_weights, attention_weights,
           global_attention_weight):
    global LAST_EXEC_NS
    from concourse import bass_utils

    nc = _get_program()
    wt, kv, w8 = _make_weights(depthwise_weights, pointwise_weights,
                               attention_weights, global_attention_weight)
    shards = _make_shards(x)
    shards8 = _make_shards8(shards)
    in_maps = [{"xs": shards[i], "wt": wt, "kv": kv,
                "x8": shards8[i], "w8": w8} for i in range(N_CORES)]

    res = bass_utils.run_bass_kernel_spmd(
        nc, in_maps, core_ids=list(range(N_CORES)), trace=False
    )
    LAST_EXEC_NS = res.exec_time_ns

    out = np.empty((B, C, H, W), np.float32)
    for i in range(N_CORES):
        p, h = divmod(i, 2)
        o = np.asarray(res.results[i]["out"]).astype(np.float32)
        o = o.reshape(2, C, HH, W)
        out[2 * p:2 * p + 2, :, HH * h:HH * h + HH, :] = o
    return out


# revision 5
# speedup vs baseline: 1.0040x; 1.0040x over previous
"""Trainium2 Bass kernel for nn_DepthwiseMultiKernelAttention.

out = softmax_rows(G) @ P @ softmax_rows(A) @ depthwise3x3(x, K) folds to
out = sum_s M9[s] @ shift_s(x) with M9[s] = M*diag(K[:,s]), M = Gsm@P@Asm
(all host-folded). 8 cores: core i = (sample pair i//2, row half i%2);
2 samples x 64 ch packed in the 128 SBUF partitions, host pre-pads halos.

f16 end-to-end (DMA ~52us on the shared device; DVE 2x/4x perf modes;
rel err ~7e-4). Per 16-row chunk, per 8-row group:
  PE:   taps 0,1,2,8 as blockdiag matmuls per 2-row psum tile + 2 mix folds
  DVE:  taps 3,4,5 as ts,ts,tt,ts,tt chain (ts@4x, tt@2x) + ts tap 7
  Act:  tap 6 via activation(scale) + psum->sbuf f32->f16 drains (2-row)
  DMA:  SWDGE accumulate (t6 += t7) merges chain2 on the DMA engines --
        Pool pays only the ~1us descriptor issue, transfers pipeline on
        the half-idle DMA device (Pool tensor_add was the serial spine)
Vector chains run 2 chunks ahead of PE; folds deferred 4 tiles; drains
attach to their tile's fold (emission order = tile-framework dependency
order: a drain emitted before its fold reads pre-fold psum) and release
1-bank psum slots. Stores ride SP HWDGE per chunk; tail chunk drains
and stores at 2-row grain alternating Act/DVE.
TimelineSim (the graded metric): ~92.7us vs 103.5us baseline.
"""

import numpy as np

B, C, H, W = 8, 64, 256, 256
N_CORES = 8
HH = H // 2
PR, PC = HH + 2, W + 2
RPC = 16           # rows per chunk
N_CHUNKS = HH // RPC
GR = 8             # rows per vector group / psum tile
RPP = 2            # rows per matmul (one psum bank)
SHIFTS = [(dy, dx) for dy in range(3) for dx in range(3)]

PE_TAPS = (0, 1, 2)      # always-PE taps
DVE_TAPS = (3, 4, 5)     # DVE ts/tt chain
ACT_TAP = 6              # Act activation tap (chain2 base)
POOL_TAP = 7             # Pool stt tap (merges Act product)
TAP8 = 8

# per-group assignment of tap 8: 'pe' | 'pool' | 'dve'
N_GROUPS = N_CHUNKS * (RPC // GR)  # 16
DEFAULT_TAP8 = tuple(('pe', 'pool')[g % 2] for g in range(N_GROUPS))


def tap8_mix(pat):
    """Cyclic tap8 schedule from a pattern string like 'pe,pool,dve,pool'."""
    names = pat.split(',')
    return tuple(names[g % len(names)] for g in range(N_GROUPS))

WSLOT = {0: 0, 1: 1, 2: 2, TAP8: 3, ACT_TAP: 5, POOL_TAP: 6}
MIX_SLOT = 4
NW = 7

LAST_EXEC_NS = None
_PROGRAM = None


BEST_TAP8 = tap8_mix('pe')
DR_PAIRS = ((0, 1),)


def _build_program(tap8=BEST_TAP8, defer=4, x_bufs=4, o_bufs=2,
                   acc_bufs=6, tmp_bufs=2, t6_bufs=6, warmup_mm=4,
                   split_first=True, store_eng='sync', drain_defer=2,
                   ahead=2, prefetch_at='top', chain2_pe=(0,),
                   tail_dve_drains=True, chains16=False, ps_rows=2,
                   halo_dma=False, warmup_memset=True, first_piece=5,
                   chain2_dve=(), merge_dma=(), merge67='dma',
                   chain2_16=False, fold_merge=None, split_wt=False, act16=False,
                   split_chunks=(1,), dr_pairs=(), tap7_pe=False,
                   store_halves=False):
    import concourse.mybir as mybir
    import concourse.tile as tile
    from concourse import bacc

    f32 = mybir.dt.float32
    f16 = mybir.dt.float16
    mult = mybir.AluOpType.mult
    add = mybir.AluOpType.add
    Copy = mybir.ActivationFunctionType.Copy

    nc = bacc.Bacc("TRN2", target_bir_lowering=False, debug=False,
                   num_devices=N_CORES)
    x_d = nc.dram_tensor("xs", [128, PR, PC], f16, kind="ExternalInput").ap()
    w_d = nc.dram_tensor("wt", [128, NW, 128], f16, kind="ExternalInput").ap()
    k_d = nc.dram_tensor("kv", [128, 9], f32, kind="ExternalInput").ap()
    f8 = mybir.dt.float8e4
    x8_d = nc.dram_tensor("x8", [128, PR, PC], f8, kind="ExternalInput").ap()
    w8_d = nc.dram_tensor("w8", [128, 2, 2, 128], f8,
                          kind="ExternalInput").ap()
    o_d = nc.dram_tensor("out", [128, HH, W], f16, kind="ExternalOutput").ap()

    with tile.TileContext(nc) as tc:
        with (
            tc.tile_pool(name="wpool", bufs=1) as wpool,
            tc.tile_pool(name="xpool", bufs=x_bufs) as xpool,
            tc.tile_pool(name="ppool", bufs=16 // ps_rows,
                         space="PSUM") as ppool,
            tc.tile_pool(name="opool", bufs=o_bufs) as opool,
            tc.tile_pool(name="vpool", bufs=2) as vpool,
        ):
            # PE warmup: fills initial DMA wait so p-state is ramped.
            # warmup matmuls read uninitialized SBUF: values never escape
            # (every real accumulation group opens with start=True)
            scratch = wpool.tile([128, 512], f32, tag="scratch")
            if warmup_memset == 'dve':
                nc.vector.memset(scratch, 0.0)
            elif warmup_memset:
                nc.gpsimd.memset(scratch, 0.0)
            sc16 = scratch.bitcast(mybir.dt.bfloat16)
            wps = ppool.tile([128, ps_rows, W], f32, name="ps", tag="ps")
            for _ in range(warmup_mm):
                nc.tensor.matmul(wps[:, :2, :],
                                 lhsT=sc16[:, :128], rhs=sc16[:, :512],
                                 start=True, stop=True)

            wt = wpool.tile([128, NW, 128], f16)
            if split_wt:
                # early slots first so the first tap matmuls aren't gated on
                # the full weight block
                nc.scalar.dma_start(out=wt[:, :3, :], in_=w_d[:, :3, :])
                nc.scalar.dma_start(out=wt[:, 3:, :], in_=w_d[:, 3:, :])
            else:
                nc.scalar.dma_start(out=wt, in_=w_d)
            kv = wpool.tile([128, 9], f32)
            nc.scalar.dma_start(out=kv, in_=k_d)
            w8t = None
            if dr_pairs:
                w8t = wpool.tile([128, 2, 2, 128], f8)
                nc.scalar.dma_start(out=w8t, in_=w8_d)

            first = xpool.tile([128, RPC + 2, PC], f16, name="xt", tag="xt")
            x8ts = {}
            if dr_pairs:
                fx8 = xpool.tile([128, RPC + 2, PC], f8, name="x8t", tag="x8t")
                nc.scalar.dma_start(out=fx8[:, :10, :],
                                     in_=x8_d[:, :10, :])
                nc.scalar.dma_start(out=fx8[:, 10:, :],
                                    in_=x8_d[:, 10:RPC + 2, :])
                x8ts[0] = fx8
            first_tail_dma = None
            if split_first:
                r = 0
                while r < RPC + 2:
                    e = min(r + first_piece, RPC + 2)
                    if RPC + 2 - e < 4:
                        e = RPC + 2
                    nc.sync.dma_start(out=first[:, r:e, :],
                                      in_=x_d[:, r:e, :])
                    r = e
            else:
                nc.sync.dma_start(out=first, in_=x_d[:, :RPC + 2, :])

            # deferred PE folds: (ps_slice, acc1_sl, acc2_sl)
            pending = []
            # deferred Act drains: (chunk, ot_slice, ps8)
            drains = []
            drained = {}   # chunk -> number of drains emitted
            store_info = {}  # chunk -> (dram_slice, ot)
            store_eng_obj = None  # set below

            def flush_folds(limit=0):
                # a tile's drain is only queued once its folds are emitted —
                # otherwise the tile framework orders the drain BEFORE the
                # fold (it reads the psum's pre-fold state): wrong results
                while len(pending) > limit:
                    ps2, a1, a2, drain_info = pending.pop(0)
                    nc.tensor.matmul(ps2, lhsT=wt[:, MIX_SLOT, :], rhs=a1,
                                     start=False, stop=a2 is None,
                                     skip_group_check=bool(dr_pairs))
                    if a2 is not None:
                        nc.tensor.matmul(ps2, lhsT=wt[:, MIX_SLOT, :], rhs=a2,
                                         start=False, stop=True,
                                         skip_group_check=bool(dr_pairs))
                    if drain_info is not None:
                        drains.append(drain_info)

            def flush_drains(limit=0):
                while len(drains) > limit:
                    ck, dst, ps8 = drains.pop(0)
                    nc.scalar.copy(out=dst, in_=ps8)
                    drained[ck] = drained.get(ck, 0) + 1
                    half = RPC // ps_rows // 2
                    if store_halves and drained[ck] == half:
                        dsl, ot_t = store_info[ck]
                        store_eng_obj.dma_start(
                            out=dsl[:, :RPC // 2, :],
                            in_=ot_t[:, :RPC // 2, :])
                    elif drained[ck] == RPC // ps_rows:
                        dsl, ot_t = store_info.pop(ck)
                        if store_halves:
                            store_eng_obj.dma_start(
                                out=dsl[:, RPC // 2:, :],
                                in_=ot_t[:, RPC // 2:, :])
                        else:
                            store_eng_obj.dma_start(out=dsl, in_=ot_t)

            def drain_fine(ck, ot_t, ps8, rg, done_tiles):
                """2-row drains + store slices for the tail (latency);
                alternate Act/DVE so the tail is not Act-serial."""
                for t in done_tiles:
                    sl = slice(rg + RPP * t, rg + RPP * (t + 1))
                    use_dve = (tail_dve_drains is True and
                               (rg // RPP + t) % 2 == 1)
                    if tail_dve_drains == 'last' and rg + RPP * t >= RPC - 6:
                        use_dve = (rg // RPP + t) % 2 == 1
                    if use_dve:
                        nc.vector.tensor_copy(ot_t[:, sl, :],
                                              ps8[:, RPP * t:RPP * (t + 1), :])
                    else:
                        nc.scalar.copy(out=ot_t[:, sl, :],
                                       in_=ps8[:, RPP * t:RPP * (t + 1), :])
                    r0 = ck * RPC
                    store_eng_obj.dma_start(
                        out=o_d[:, r0 + sl.start:r0 + sl.stop, :],
                        in_=ot_t[:, sl, :])

            def emit_chains16(chunk, xt_t):
                """Whole-chunk (16-row) vector chains: fewer, bigger ops."""
                g = chunk * (RPC // GR)  # use first group's tap8 assignment

                def xs16(si):
                    dy, dx = SHIFTS[si]
                    return xt_t[:, dy:dy + RPC, dx:dx + W]

                t3 = vpool.tile([128, RPC, W], f16, name="t3", tag="t3",
                                bufs=tmp_bufs)
                nc.vector.tensor_scalar(t3, xs16(DVE_TAPS[0]),
                                        kv[:, DVE_TAPS[0]:DVE_TAPS[0] + 1],
                                        None, mult)
                t4 = vpool.tile([128, RPC, W], f16, name="t4", tag="t4",
                                bufs=tmp_bufs)
                nc.vector.tensor_scalar(t4, xs16(DVE_TAPS[1]),
                                        kv[:, DVE_TAPS[1]:DVE_TAPS[1] + 1],
                                        None, mult)
                acc1 = vpool.tile([128, RPC, W], f16, name="acc1", tag="acc1",
                                  bufs=acc_bufs)
                nc.vector.tensor_tensor(acc1, t3, t4, add)
                t5 = vpool.tile([128, RPC, W], f16, name="t5", tag="t5",
                                bufs=tmp_bufs)
                nc.vector.tensor_scalar(t5, xs16(DVE_TAPS[2]),
                                        kv[:, DVE_TAPS[2]:DVE_TAPS[2] + 1],
                                        None, mult)
                nc.vector.tensor_tensor(acc1, acc1, t5, add)
                if tap8[g] == 'dve' and g not in chain2_pe:
                    t8 = vpool.tile([128, RPC, W], f16, name="t8", tag="t8",
                                    bufs=tmp_bufs)
                    nc.vector.tensor_scalar(t8, xs16(TAP8),
                                            kv[:, TAP8:TAP8 + 1], None, mult)
                    nc.vector.tensor_tensor(acc1, acc1, t8, add)
                if g in chain2_pe:
                    return {s: (acc1[:, GR * s:GR * (s + 1), :], None, True)
                            for s in range(RPC // GR)}
                t6 = vpool.tile([128, RPC, W], f16, name="t6", tag="t6",
                                bufs=t6_bufs)
                nc.scalar.activation(out=t6, in_=xs16(ACT_TAP), func=Copy,
                                     scale=kv[:, ACT_TAP:ACT_TAP + 1])
                acc2 = vpool.tile([128, RPC, W], f16, name="acc2", tag="acc2",
                                  bufs=acc_bufs)
                nc.gpsimd.scalar_tensor_tensor(
                    out=acc2, in0=xs16(POOL_TAP),
                    scalar=kv[:, POOL_TAP:POOL_TAP + 1], in1=t6,
                    op0=mult, op1=add)
                if tap8[g] == 'pool':
                    nc.gpsimd.scalar_tensor_tensor(
                        out=acc2, in0=xs16(TAP8), scalar=kv[:, TAP8:TAP8 + 1],
                        in1=acc2, op0=mult, op1=add)
                pe8 = tap8[g] == 'pe'
                return {s: (acc1[:, GR * s:GR * (s + 1), :],
                            acc2[:, GR * s:GR * (s + 1), :], pe8)
                        for s in range(RPC // GR)}

            def emit_chain2_16(chunk, xt_t):
                """Chunk-granularity chain2: act6 + ts7 + Pool tt merge over
                16 rows (amortizes per-op init overheads)."""
                def xs16(si):
                    dy, dx = SHIFTS[si]
                    return xt_t[:, dy:dy + RPC, dx:dx + W]

                t6 = vpool.tile([128, RPC, W], f16, name="t6", tag="t6",
                                bufs=3)
                nc.scalar.activation(out=t6, in_=xs16(ACT_TAP), func=Copy,
                                     scale=kv[:, ACT_TAP:ACT_TAP + 1])
                t7 = vpool.tile([128, RPC, W], f16, name="t7", tag="t7",
                                bufs=3)
                nc.vector.tensor_scalar(t7, xs16(POOL_TAP),
                                        kv[:, POOL_TAP:POOL_TAP + 1],
                                        None, mult)
                acc2 = vpool.tile([128, RPC, W], f16, name="acc2",
                                  tag="acc2", bufs=3)
                if merge67 == 'pool':
                    nc.gpsimd.tensor_add(acc2, t6, t7)
                else:
                    nc.vector.tensor_add(acc2, t6, t7)
                return acc2

            def emit_chains(chunk, xt_t):
                """Vector-engine tap chains for both groups of a chunk."""
                if chains16:
                    return emit_chains16(chunk, xt_t)
                acc2_16 = None
                t6_16 = None
                if chain2_16:
                    g0 = chunk * (RPC // GR)
                    if (g0 not in chain2_pe and g0 not in chain2_dve):
                        acc2_16 = emit_chain2_16(chunk, xt_t)
                elif act16:
                    g0 = chunk * (RPC // GR)
                    if (g0 not in chain2_pe and g0 not in chain2_dve
                            and (g0 + 1) not in chain2_pe):
                        # 16-row act6 (amortized init); per-group merges
                        dy, dx = SHIFTS[ACT_TAP]
                        t6_16 = vpool.tile([128, RPC, W], f16, name="t616",
                                           tag="t616", bufs=3)
                        nc.scalar.activation(
                            out=t6_16, in_=xt_t[:, dy:dy + RPC, dx:dx + W],
                            func=Copy, scale=kv[:, ACT_TAP:ACT_TAP + 1])
                out = {}
                for sub in range(RPC // GR):
                    g = chunk * (RPC // GR) + sub
                    rg = GR * sub

                    def xs8(si):
                        dy, dx = SHIFTS[si]
                        return xt_t[:, rg + dy:rg + dy + GR, dx:dx + W]

                    # chain1 on DVE: taps 3,4,5 (+ tap8 if assigned)
                    t3 = vpool.tile([128, GR, W], f16, name="t3", tag="t3",
                                    bufs=tmp_bufs)
                    nc.vector.tensor_scalar(t3, xs8(DVE_TAPS[0]),
                                            kv[:, DVE_TAPS[0]:DVE_TAPS[0] + 1],
                                            None, mult)
                    t4 = vpool.tile([128, GR, W], f16, name="t4", tag="t4",
                                    bufs=tmp_bufs)
                    nc.vector.tensor_scalar(t4, xs8(DVE_TAPS[1]),
                                            kv[:, DVE_TAPS[1]:DVE_TAPS[1] + 1],
                                            None, mult)
                    a34 = vpool.tile([128, GR, W], f16, name="a34", tag="a34",
                                     bufs=tmp_bufs)
                    nc.vector.tensor_tensor(a34, t3, t4, add)
                    t5 = vpool.tile([128, GR, W], f16, name="t5", tag="t5",
                                    bufs=tmp_bufs)
                    nc.vector.tensor_scalar(t5, xs8(DVE_TAPS[2]),
                                            kv[:, DVE_TAPS[2]:DVE_TAPS[2] + 1],
                                            None, mult)
                    if (tap8[g] == 'dve' and g not in chain2_pe
                            and g not in chain2_dve):
                        a345 = vpool.tile([128, GR, W], f16, name="a345",
                                          tag="a345", bufs=tmp_bufs)
                        nc.vector.tensor_tensor(a345, a34, t5, add)
                        t8 = vpool.tile([128, GR, W], f16, name="t8", tag="t8",
                                        bufs=tmp_bufs)
                        nc.vector.tensor_scalar(t8, xs8(TAP8),
                                                kv[:, TAP8:TAP8 + 1],
                                                None, mult)
                        acc1 = vpool.tile([128, GR, W], f16, name="acc1",
                                          tag="acc1", bufs=acc_bufs)
                        nc.vector.tensor_tensor(acc1, a345, t8, add)
                    else:
                        acc1 = vpool.tile([128, GR, W], f16, name="acc1",
                                          tag="acc1", bufs=acc_bufs)
                        nc.vector.tensor_tensor(acc1, a34, t5, add)

                    if g in chain2_pe:
                        # startup/tail groups: chain2 taps 6,7 ride PE (no
                        # Act/Pool latency in the critical path); tap8 per
                        # its own assignment ('dve' folds into chain1)
                        out[sub] = (acc1, None, tap8[g] != 'dve')
                        continue
                    if g in chain2_dve:
                        # chain2 folded into the DVE chain (ts+tt per tap)
                        for si in (ACT_TAP, POOL_TAP, TAP8):
                            tx = vpool.tile([128, GR, W], f16, name="tx",
                                            tag="tx", bufs=tmp_bufs)
                            nc.vector.tensor_scalar(tx, xs8(si),
                                                    kv[:, si:si + 1],
                                                    None, mult)
                            na = vpool.tile([128, GR, W], f16, name="acc1",
                                            tag="acc1", bufs=acc_bufs)
                            nc.vector.tensor_tensor(na, acc1, tx, add)
                            acc1 = na
                        out[sub] = (acc1, None, False)
                        continue
                    if acc2_16 is not None:
                        out[sub] = (acc1,
                                    acc2_16[:, rg:rg + GR, :],
                                    tap8[g] != 'dve')
                        continue
                    # chain2: Act tap6 + DVE ts tap7, merged by a Pool
                    # tensor_tensor add (Pool can't run TensorScalarPtr)
                    if t6_16 is not None:
                        t6 = t6_16[:, rg:rg + GR, :]
                    else:
                        t6 = vpool.tile([128, GR, W], f16, name="t6",
                                        tag="t6", bufs=t6_bufs)
                        nc.scalar.activation(out=t6, in_=xs8(ACT_TAP),
                                             func=Copy,
                                             scale=kv[:, ACT_TAP:ACT_TAP + 1])
                    t7 = vpool.tile([128, GR, W], f16, name="t7", tag="t7",
                                    bufs=t6_bufs)
                    nc.vector.tensor_scalar(t7, xs8(POOL_TAP),
                                            kv[:, POOL_TAP:POOL_TAP + 1],
                                            None, mult)
                    m67 = merge67
                    if merge67 == 'alt':
                        m67 = 'pool' if g % 2 == 0 else 'dve'
                    elif merge67 == 'alt4':
                        m67 = 'dve' if g % 4 == 3 else 'pool'
                    if m67 == 'dma':
                        # SWDGE accumulate: t6 += t7 on the DMA engines;
                        # Pool only pays the ~1us descriptor-gen issue
                        nc.gpsimd.dma_start(out=t6, in_=t7,
                                            accum_op=add)
                        acc2 = t6
                    elif True:
                        acc2 = vpool.tile([128, GR, W], f16, name="acc2",
                                          tag="acc2", bufs=acc_bufs)
                    if m67 == 'dma':
                        pass
                    elif m67 == 'pool':
                        nc.gpsimd.tensor_add(acc2, t6, t7)
                    else:
                        nc.vector.tensor_add(acc2, t6, t7)
                    if g in merge_dma:
                        # fold acc2 into acc1 on the DMA engines (CCE
                        # accumulate) — saves one PE mix-matmul per tile
                        nc.gpsimd.dma_start(out=acc1, in_=acc2,
                                            accum_op=add)
                        out[sub] = (acc1, None, tap8[g] != 'dve')
                    elif fold_merge and fold_merge[g]:
                        # merge acc1+acc2 on a vector engine: one PE fold
                        # per tile instead of two
                        accm = vpool.tile([128, GR, W], f16, name="accm",
                                          tag="accm", bufs=acc_bufs)
                        if fold_merge[g] == 'pool':
                            nc.gpsimd.tensor_add(accm, acc1, acc2)
                        else:
                            nc.vector.tensor_add(accm, acc1, acc2)
                        out[sub] = (accm, None, tap8[g] != 'dve')
                    else:
                        out[sub] = (acc1, acc2, tap8[g] != 'dve')
                return out

            store_eng_obj = {'sync': nc.sync, 'gpsimd': nc.gpsimd,
                             'scalar': nc.scalar}[store_eng]
            xts = {0: first}
            accs = {0: emit_chains(0, first)}

            def prefetch(c):
                if c >= N_CHUNKS or c in xts:
                    return
                r0 = c * RPC
                nxt = xpool.tile([128, RPC + 2, PC], f16, name="xt",
                                 tag="xt")
                if halo_dma:
                    # halo rows re-read from HBM (DMA has slack; frees DVE
                    # and decouples this tile from the previous one)
                    nc.sync.dma_start(out=nxt,
                                      in_=x_d[:, r0:r0 + RPC + 2, :])
                else:
                    # halo rows from previous chunk's tile (DVE 4x copy)
                    nc.vector.tensor_copy(nxt[:, 0:2, :],
                                          xts[c - 1][:, RPC:RPC + 2, :])
                    if c in split_chunks:
                        # two pieces: the chunk's chain2 (rows 2..12) can
                        # start ~1.5us before the full chunk lands
                        nc.sync.dma_start(out=nxt[:, 2:12, :],
                                          in_=x_d[:, r0 + 2:r0 + 12, :])
                        nc.sync.dma_start(out=nxt[:, 12:, :],
                                          in_=x_d[:, r0 + 12:r0 + RPC + 2, :])
                    else:
                        nc.sync.dma_start(out=nxt[:, 2:, :],
                                          in_=x_d[:, r0 + 2:r0 + RPC + 2, :])
                if dr_pairs:
                    nx8 = xpool.tile([128, RPC + 2, PC], f8, name="x8t",
                                     tag="x8t")
                    nc.scalar.dma_start(out=nx8,
                                         in_=x8_d[:, r0:r0 + RPC + 2, :])
                    x8ts[c] = nx8
                xts[c] = nxt
                accs[c] = emit_chains(c, nxt)

            for c in range(1, ahead):
                prefetch(c)
                if c == 1 and first_tail_dma is not None:
                    first_tail_dma()
                    first_tail_dma = None
            if first_tail_dma is not None:
                first_tail_dma()
                first_tail_dma = None
            for chunk in range(N_CHUNKS):
                xt = xts[chunk]
                if prefetch_at == 'top':
                    prefetch(chunk + ahead)

                ot = opool.tile([128, RPC, W], f16)
                last = chunk == N_CHUNKS - 1
                if last:
                    flush_drains(0)
                else:
                    store_info[chunk] = (
                        o_d[:, chunk * RPC:chunk * RPC + RPC, :], ot)
                # PE loop over 2-row tiles, psum tiles of ps_rows rows
                for sub in range(RPC // GR):
                    rg = GR * sub
                    acc1, acc2, pe_tap8 = accs[chunk][sub]
                    for pt in range(GR // ps_rows):
                        rp = rg + ps_rows * pt
                        psT = ppool.tile([128, ps_rows, W], f32, name="ps",
                                         tag="ps")
                        for t in range(ps_rows // RPP):
                            rr = rp + RPP * t

                            def xs2(si):
                                dy, dx = SHIFTS[si]
                                return xt[:, rr + dy:rr + dy + RPP,
                                          dx:dx + W]

                            ps2 = psT[:, RPP * t:RPP * (t + 1), :]
                            pe_sis = list(PE_TAPS)
                            if acc2 is None:
                                pe_sis += [ACT_TAP, POOL_TAP]
                            if pe_tap8:
                                pe_sis.append(TAP8)
                            dr_cov = set()
                            for (sa, sb) in dr_pairs:
                                dr_cov |= {sa, sb}
                            pe_sis = [s_ for s_ in pe_sis
                                      if s_ not in dr_cov]
                            # fp8 DoubleRow: one half-cost matmul per psum
                            # row computes BOTH taps of a pair (the pair dim
                            # is a stride-delta view into the fp8 x copy)
                            x8t = x8ts.get(chunk)
                            for i, si in enumerate(pe_sis):
                                nc.tensor.matmul(ps2,
                                                 lhsT=wt[:, WSLOT[si], :],
                                                 rhs=xs2(si),
                                                 start=(i == 0),
                                                 stop=False,
                                                 skip_group_check=bool(
                                                     dr_pairs))
                            for p, (sa, sb) in enumerate(dr_pairs):
                                dya, dxa = SHIFTS[sa]
                                dyb, dxb = SHIFTS[sb]
                                delta = (dyb - dya) * PC + (dxb - dxa)
                                for r_ in range(RPP):
                                    base = x8t[:, rr + dya + r_,
                                               dxa:dxa + W]
                                    rhs8 = type(base)(
                                        tensor=base.tensor,
                                        offset=base.offset,
                                        ap=[base.ap[0], [delta, 2], [1, W]])
                                    nc.tensor.matmul(
                                        psT[:, RPP * t + r_:
                                            RPP * t + r_ + 1, :],
                                        lhsT=w8t[:, p, :, :], rhs=rhs8,
                                        start=False, stop=False,
                                        perf_mode=(
                                            mybir.MatmulPerfMode.DoubleRow),
                                        skip_group_check=True)
                            sl = slice(rr - rg, rr - rg + RPP)
                            a2sl = None if acc2 is None else acc2[:, sl, :]
                            if last:
                                flush_folds(limit=0)
                                nc.tensor.matmul(ps2, lhsT=wt[:, MIX_SLOT, :],
                                                 rhs=acc1[:, sl, :],
                                                 start=False,
                                                 stop=a2sl is None,
                                                 skip_group_check=bool(
                                                     dr_pairs))
                                if a2sl is not None:
                                    nc.tensor.matmul(
                                        ps2, lhsT=wt[:, MIX_SLOT, :],
                                        rhs=a2sl, start=False, stop=True,
                                        skip_group_check=bool(dr_pairs))
                                drain_fine(chunk, ot, psT, rp, [t])
                            else:
                                flush_folds(limit=defer)
                                dinfo = None
                                if t == ps_rows // RPP - 1:
                                    dinfo = (chunk,
                                             ot[:, rp:rp + ps_rows, :], psT)
                                pending.append(
                                    (ps2, acc1[:, sl, :], a2sl, dinfo))
                        if not last:
                            flush_drains(limit=drain_defer)
                if prefetch_at == 'bottom':
                    prefetch(chunk + ahead)
            flush_folds(0)
            flush_drains(0)
    nc.compile()
    return nc


def _get_program():
    global _PROGRAM
    if _PROGRAM is None:
        _PROGRAM = _build_program()
    return _PROGRAM


def _softmax_rows(a):
    a = a.astype(np.float64)
    a = np.exp(a - a.max(axis=1, keepdims=True))
    return a / a.sum(axis=1, keepdims=True)


def _make_weights(depthwise_weights, pointwise_weights, attention_weights,
                  global_attention_weight):
    A = _softmax_rows(np.asarray(attention_weights))
    G = _softmax_rows(np.asarray(global_attention_weight))
    P = np.asarray(pointwise_weights)[:, :, 0, 0].astype(np.float64)
    M = G @ P @ A
    Kdw = np.asarray(depthwise_weights)[:, 0].astype(np.float64)  # (64,3,3)
    wt = np.zeros((128, NW, 128), np.float16)
    for si in list(WSLOT):
        dy, dx = SHIFTS[si]
        blk = (M.T * Kdw[:, dy, dx][:, None]).astype(np.float16)
        wt[:C, WSLOT[si], :C] = blk
        wt[C:, WSLOT[si], C:] = blk
    mixT = M.T.astype(np.float16)
    wt[:C, MIX_SLOT, :C] = mixT
    wt[C:, MIX_SLOT, C:] = mixT
    kva = np.empty((128, 9), np.float32)
    for si, (dy, dx) in enumerate(SHIFTS):
        kva[:C, si] = Kdw[:, dy, dx]
        kva[C:, si] = Kdw[:, dy, dx]
    # fp8 DoubleRow pair weights
    import concourse.mybir as mybir
    f8np = mybir.dt.np(mybir.dt.float8e4)
    w8 = np.zeros((128, 2, 2, 128), f8np)
    for p, (sa, sb) in enumerate(DR_PAIRS):
        for i, si in enumerate((sa, sb)):
            dy, dx = SHIFTS[si]
            blk = (M.T * Kdw[:, dy, dx][:, None]).astype(f8np)
            w8[:C, p, i, :C] = blk
            w8[C:, p, i, C:] = blk
    return wt, kva, w8


def _make_shards(x):
    x = np.asarray(x, dtype=np.float32)
    shards = []
    for i in range(N_CORES):
        p, h = divmod(i, 2)
        buf = np.zeros((2, C, PR, PC), np.float16)
        r0 = HH * h - 1
        r1 = HH * h + HH + 1
        sr0, sr1 = max(r0, 0), min(r1, H)
        buf[:, :, sr0 - r0:sr1 - r0, 1:1 + W] = (
            x[2 * p:2 * p + 2, :, sr0:sr1, :].astype(np.float16))
        shards.append(buf.reshape(128, PR, PC))
    return shards


def _make_shards8(shards):
    import concourse.mybir as mybir
    f8np = mybir.dt.np(mybir.dt.float8e4)
    return [s.astype(f8np) for s in shards]


def kernel(x, depthwise_weights, pointwise_weights, attention_weights,
           global_attention_weight):
    global LAST_EXEC_NS
    from concourse import bass_utils

    nc = _get_program()
    wt, kv, w8 = _make_weights(depthwise_weights, pointwise_weights,
                               attention_weights, global_attention_weight)
    shards = _make_shards(x)
    shards8 = _make_shards8(shards)
    in_maps = [{"xs": shards[i], "wt": wt, "kv": kv,
                "x8": shards8[i], "w8": w8} for i in range(N_CORES)]

    res = bass_utils.run_bass_kernel_spmd(
        nc, in_maps, core_ids=list(range(N_CORES)), trace=False
    )
    LAST_EXEC_NS = res.exec_time_ns

    out = np.empty((B, C, H, W), np.float32)
    for i in range(N_CORES):
        p, h = divmod(i, 2)
        o = np.asarray(res.results[i]["out"]).astype(np.float32)
        o = o.reshape(2, C, HH, W)
        out[2 * p:2 * p + 2, :, HH * h:HH * h + HH, :] = o
    return out
